# revision 65
# baseline (speedup 1.0000x reference)
"""Trainium2 Bass kernel for nn_Attention_40570261078258.

Computes, for x:(8,128,64,64), Wq/Wk/Wv:(128,128), bq/bk/bv:(128,):
    xf = x.reshape(N, C, L);  L = 4096
    q/k/v = W @ xf + b                  -> (N, L, C) logical
    scores = q @ k^T / sqrt(C)          -> (N, L, L)
    attn = softmax(scores, axis=0)      # over the BATCH axis (torch legacy dim=0)
    out = attn @ v                      -> (N, L, C)
    return x + out.reshape(N, C, H, W)  # reinterpreting (L,C) memory as (C,H,W)

Sharding: the softmax couples all batch elements at each (l, m) pair, so
batch-parallel would need a 64MB denominator all-reduce. Instead we shard the
query dim L across the 8 cores: each core handles l in [d*512, (d+1)*512) for
ALL batch elements, making the softmax entirely local (no collectives).
Each core redundantly computes k/v for all of L (cheap vs. attention).

SPMD: all cores run the identical graph; the per-core slice is selected by the
host passing a per-core q-input slice (xq). The device returns the attention
output in (c,l)-major tiles; the host reinterleaves and adds the residual.
"""

import math

import numpy as np

import concourse.bacc as bacc
import concourse.bass as bass
import concourse.mybir as mybir
import concourse.tile as tile
from concourse import masks
from concourse.bass_utils import run_bass_kernel_spmd

N, C, H, W = 8, 128, 64, 64
L = H * W            # 4096 pixels
NCORES = 8
LSH = L // NCORES    # 512 query positions per core
NLH = 2              # l-halves per core
LHW = LSH // NLH     # 256 l per half
NMT = L // 128       # 32 key/value tiles of 128

FP = mybir.dt.float32
FR = mybir.dt.float32r
BF = mybir.dt.bfloat16
AF = mybir.ActivationFunctionType

SKEW = 3           # m-tiles of lookahead between scores and softmax/AV
MUL_POOL_GROUPS = 4   # of the 8 normalize-mul batch groups, how many on gpsimd
REPEAT = 1         # benchmarking: emit the attention phase this many times
MUL_FLAT = False   # normalize-mul as 8 plain 2D ops instead of 3D broadcast
BENCH_INTERNAL = False  # benchmarking: x + out in internal DRAM (no transfer)

# Set by test harness to capture a profile.
TRACE = False
LAST_RESULTS = None


def build():
    nc = bacc.Bacc(
        "TRN2",
        target_bir_lowering=False,
        debug=False,
        enable_asserts=True,
        num_devices=NCORES,
    )

    # x and the transposed weights are declared float32r (same bits as f32)
    # so the projection matmuls run at full PE rate without a bf16 pre-cast.
    if not BENCH_INTERNAL:
        xk = nc.dram_tensor("xk", [N, C, L], FR, kind="ExternalInput").ap()
        xq = nc.dram_tensor("xq", [N, C, LSH], FR, kind="ExternalInput").ap()
    else:
        xk = nc.dram_tensor("xk_i", [N, C, L], FR, kind="Internal").ap()
        xq = nc.dram_tensor("xq_i", [N, C, LSH], FR, kind="Internal").ap()
    wq = nc.dram_tensor("wq", [C, C], FP, kind="ExternalInput").ap()
    wk = nc.dram_tensor("wk", [C, C], FP, kind="ExternalInput").ap()
    wv = nc.dram_tensor("wv", [C, C], FP, kind="ExternalInput").ap()
    bq = nc.dram_tensor("bq", [C, 1], FP, kind="ExternalInput").ap()
    bk = nc.dram_tensor("bk", [C, 1], FP, kind="ExternalInput").ap()
    bv = nc.dram_tensor("bv", [1, C], FP, kind="ExternalInput").ap()
    # Attention output in (c, l)-major layout; the host does the cheap
    # (l,c) reinterleave + residual add (pure glue, 0.4% of the FLOPs).
    if not BENCH_INTERNAL:
        out = nc.dram_tensor(
            "out", [N, NLH, C, LHW], FP, kind="ExternalOutput"
        ).ap()
    else:
        out = nc.dram_tensor(
            "out_i", [N, NLH, C, LHW], FP, kind="Internal"
        ).ap()
        tok = nc.dram_tensor("tok", [1, 4], FP, kind="ExternalOutput").ap()

    with tile.TileContext(nc) as tc:
        if BENCH_INTERNAL:
            # Zero the internal x so exp() sees sane values; one-time cost,
            # constant across variants (cancels in the repeat slope).
            with tc.tile_pool(name="zinit", bufs=1) as zp:
                zt = zp.tile([128, 2048], FP, tag="z0")
                nc.vector.memset(zt[:], 0.0)
                xkf = xk.rearrange("n c l -> (n c) l").rearrange(
                    "(b p) l -> b p l", p=128
                )
                for b in range(xkf.shape[0]):
                    for c0 in range(0, xkf.shape[2], 2048):
                        nc.sync.dma_start(
                            xkf[b, :, c0 : c0 + 2048].bitcast(FP), zt[:]
                        )
                xqf = xq.rearrange("n c l -> (n c) l").rearrange(
                    "(b p) l -> b p l", p=128
                )
                for b in range(xqf.shape[0]):
                    nc.sync.dma_start(xqf[b].bitcast(FP), zt[:, : xqf.shape[2]])
                nc.sync.dma_start(tok, zt[0:1, 0:4])
        _emit(nc, tc, xk, xq, wq, wk, wv, bq, bk, bv, out)

    nc.compile()
    return nc


def _emit(nc, tc, xk, xq, wq, wk, wv, bq, bk, bv, out):
    from contextlib import ExitStack

    with ExitStack() as ctx:
        cpool = ctx.enter_context(tc.tile_pool(name="const", bufs=1))
        resid = ctx.enter_context(tc.tile_pool(name="resident", bufs=1))

        # --- constants -----------------------------------------------------
        ident = cpool.tile([128, 128], FP, tag="ident")
        masks.make_identity(nc, ident[:])

        bq_t = cpool.tile([C, 1], FP, tag="bq")
        nc.sync.dma_start(bq_t[:], bq)
        bk_t = cpool.tile([C, 1], FP, tag="bk")
        nc.sync.dma_start(bk_t[:], bk)
        bv_f = cpool.tile([1, C], FP, tag="bvf")
        nc.sync.dma_start(bv_f[:], bv)
        ones_row = cpool.tile([1, C], FP, tag="ones")
        nc.vector.memset(ones_row[:], 1.0)
        # bv replicated across partitions (rank-1 ones @ bv matmul)
        bv_rep = cpool.tile([128, C], FP, tag="bvrep")

        # Transposed f32 weights: WT[c, o] = W[o, c]; projections run as
        # float32r matmuls (full PE rate at free dim >= 256, ~fp32 precision,
        # and no bf16 pre-cast of x needed).
        wT = {}
        with (
            tc.tile_pool(name="wtmp", bufs=2) as wtmp_pool,
            tc.tile_pool(name="wpsum", bufs=2, space="PSUM") as wpsum_pool,
        ):
            for name, wap in (("q", wq), ("k", wk), ("v", wv)):
                wf = wtmp_pool.tile([C, C], FP, tag="wf")
                nc.sync.dma_start(wf[:], wap)
                ps = wpsum_pool.tile([128, 128], FP, tag="wps")
                nc.tensor.transpose(ps[:], wf[:], ident[:])
                wt = cpool.tile([C, C], FR, tag=f"w{name}T")
                nc.scalar.copy(wt[:], ps[:])
                wT[name] = wt
            pb = wpsum_pool.tile([128, C], FP, tag="wps")
            nc.tensor.matmul(pb[:], ones_row[:], bv_f[:], start=True, stop=True)
            nc.vector.tensor_copy(bv_rep[:], pb[:])
        # WvT padded to 256 columns of zeros so the float32r vT matmuls hit
        # the >=256 free-dim full-rate path (junk half never read).
        wvpad = cpool.tile([C, 2 * C], FR, tag="wvpad")
        zpad = cpool.tile([C, 2 * C], FP, tag="zpad")
        nc.vector.memset(zpad[:], 0.0)
        nc.vector.tensor_copy(wvpad[:], zpad[:])
        nc.vector.tensor_copy(wvpad[:, 0:C], wT["v"][:])

        # --- resident activations -----------------------------------------
        # q_sb[n]: (c, l) for this core's l-slice;  k_sb[n]: (c, m) full L;
        # vT_sb[n]: (m % 128, 32*128) i.e. 32 chunks of (m,c), all bf16.
        q_sb = [
            resid.tile([C, LSH], BF, tag=f"q{n}", name=f"q_sb{n}") for n in range(N)
        ]
        k_sb = [
            resid.tile([C, L], BF, tag=f"k{n}", name=f"k_sb{n}") for n in range(N)
        ]
        vT_sb = [
            resid.tile([128, NMT * C], BF, tag=f"v{n}", name=f"vT_sb{n}")
            for n in range(N)
        ]

        # --- phase 1: projections (float32r matmuls straight from f32r x) ---
        wqT_r = wT["q"][:]
        wkT_r = wT["k"][:]
        wvpad_r = wvpad[:]
        with (
            tc.tile_pool(name="xin", bufs=3) as xin_pool,
            tc.tile_pool(name="pj", bufs=2, space="PSUM") as pj_psum,
            tc.tile_pool(name="pv", bufs=2, space="PSUM") as pv_psum,
        ):
            for n in range(N):
                # q from the per-core slice
                xt = xin_pool.tile([C, LSH], FR, tag="x")
                nc.sync.dma_start(xt[:], xq[n])
                pq = pj_psum.tile([128, 1024], FP, tag="pj")
                nc.tensor.matmul(
                    pq[:, 0:512], wqT_r, xt[:], start=True, stop=True
                )
                nc.scalar.activation(
                    q_sb[n][:], pq[:, 0:512], AF.Identity, bias=bq_t[:]
                )

                for bch in range(L // 1024):
                    xt = xin_pool.tile([C, 1024], FR, tag="x")
                    nc.sync.dma_start(
                        xt[:], xk[n, :, bch * 1024 : (bch + 1) * 1024]
                    )
                    xr_ = xt[:]

                    # Both 512-wide k matmuls land in one 2-bank PSUM tile so
                    # the bias-adding eviction runs as a single 1024-wide op.
                    pk = pj_psum.tile([128, 1024], FP, tag="pj")
                    for half in range(2):
                        nc.tensor.matmul(
                            pk[:, half * 512 : (half + 1) * 512],
                            wkT_r,
                            xr_[:, half * 512 : (half + 1) * 512],
                            start=True,
                            stop=True,
                        )
                    nc.scalar.activation(
                        k_sb[n][:, bch * 1024 : (bch + 1) * 1024],
                        pk[:],
                        AF.Identity,
                        bias=bk_t[:],
                    )

                    for half in range(2):
                        ch = 2 * bch + half
                        # vT chunks: out[m,c] = sum_c' x[c',m] WvT[c',c] + bv[c]
                        # Each 128-m sub-tile occupies a 256-wide PSUM slice
                        # (f32r full-rate needs >=256 free; upper half junk).
                        # One group per 2KB bank: `start` only on the first
                        # matmul into each bank.
                        pv = pv_psum.tile([128, 1024], FP, tag="pv")
                        for sub in range(4):
                            sl = slice(sub * 256, sub * 256 + 256)
                            nc.tensor.matmul(
                                pv[:, sl],
                                xr_[:, half * 512 + sub * 128 :
                                    half * 512 + (sub + 1) * 128],
                                wvpad_r,
                                start=(sub % 2 == 0),
                                stop=(sub % 2 == 1),
                            )
                        # DVE (idle during phase 1) takes the vT eviction;
                        # strided 3D read picks the real 128 of each 256, and
                        # the bv bias rides along via the broadcast add.
                        nc.vector.scalar_tensor_tensor(
                            vT_sb[n][:, ch * 512 : (ch + 1) * 512].rearrange(
                                "p (s c) -> p s c", s=4
                            ),
                            pv[:].rearrange("p (s c2) -> p s c2", s=4)[
                                :, :, 0:128
                            ],
                            1.0,
                            bv_rep[:].unsqueeze(1).broadcast_to((128, 4, C)),
                            mybir.AluOpType.mult,
                            mybir.AluOpType.add,
                        )

        # --- phase 2: attention with softmax over batch --------------------
        inv_sqrt_c = 1.0 / math.sqrt(C)
        with (
            tc.tile_pool(name="scp", bufs=2, space="PSUM") as sc_psum,
            tc.tile_pool(name="avp", bufs=2, space="PSUM") as av_psum,
            tc.tile_pool(name="soft", bufs=1) as soft_pool,
            tc.tile_pool(name="ost", bufs=1) as ost_pool,
        ):
            def emit_epilogue(avp_prev, lh_prev):
                # Evict PSUM accumulators (freeing the av slots for the next
                # l-half) straight to DRAM in (c, l)-major layout.
                for n in range(N):
                    j, i = n // 4, n % 4
                    ob = ost_pool.tile([128, LHW], FP, tag="ob", bufs=4)
                    nc.vector.tensor_copy(
                        ob[:], avp_prev[j][:, i * LHW : (i + 1) * LHW]
                    )
                    nc.sync.dma_start(out[n, lh_prev], ob[:])

            pend = {}   # (lh, mt) -> E tile (128, 8n x 256l)
            avps = {}   # lh -> accumulator tiles

            def emit_scores(lh, mt):
                l0 = lh * LHW
                e = soft_pool.tile([128, 2048], BF, tag="E", bufs=SKEW + 3)
                for j in range(2):
                    ps = sc_psum.tile([128, 1024], FP, tag="sc")
                    for i in range(4):
                        n = 4 * j + i
                        nc.tensor.matmul(
                            ps[:, i * LHW : (i + 1) * LHW],
                            k_sb[n][:, mt * 128 : (mt + 1) * 128],
                            q_sb[n][:, l0 : l0 + LHW],
                            start=True,
                            stop=True,
                        )
                    nc.scalar.activation(
                        e[:, j * 1024 : (j + 1) * 1024],
                        ps[:],
                        AF.Exp,
                        scale=inv_sqrt_c,
                    )
                pend[(lh, mt)] = e

            def emit_soft_av(lh, mt):
                if mt == 0:
                    # Two (c, 4n x 256l) accumulators, 2 PSUM banks each;
                    # group start/stop is per 2KB bank.
                    avps[lh] = [
                        av_psum.tile(
                            [128, 1024], FP, tag="av", name=f"avp{lh}_{j}"
                        )
                        for j in range(2)
                    ]
                avp = avps[lh]
                e = pend.pop((lh, mt))
                if True:
                    s1 = soft_pool.tile([128, 1024], BF, tag="zt1", bufs=3)
                    nc.vector.tensor_add(s1[:], e[:, 0:1024], e[:, 1024:2048])
                    s2 = soft_pool.tile([128, 512], BF, tag="zt2", bufs=3)
                    nc.vector.tensor_add(s2[:], s1[:, 0:512], s1[:, 512:1024])
                    zr = soft_pool.tile([128, LHW], BF, tag="zr", bufs=4)
                    nc.vector.tensor_add(zr[:], s2[:, 0:LHW], s2[:, LHW : 2 * LHW])
                    r = soft_pool.tile([128, LHW], BF, tag="r", bufs=4)
                    with nc.allow_low_precision(
                        "softmax denom is a sum of 8 O(1..500) exps; bf16 ok"
                    ):
                        nc.vector.reciprocal(r[:], zr[:])
                    # attn[n] = E[n] * (1/Z) via stride-0 broadcast of r along
                    # the batch-group dim; n0-3 on DVE, n4-7 on the otherwise
                    # idle gpsimd (each half feeds its own AV matmuls, so the
                    # slower engine's latency pipelines away).
                    a = soft_pool.tile([128, 2048], BF, tag="A", bufs=SKEW + 1)
                    gd = 8 - MUL_POOL_GROUPS
                    if MUL_FLAT:
                        for g in range(8):
                            eng = nc.vector if g < gd else nc.gpsimd
                            eng.tensor_mul(
                                a[:, g * LHW : (g + 1) * LHW],
                                e[:, g * LHW : (g + 1) * LHW],
                                r[:],
                            )
                    elif gd:
                        nc.vector.tensor_mul(
                            a[:, : gd * LHW].rearrange("p (g l) -> p g l", g=gd),
                            e[:, : gd * LHW].rearrange("p (g l) -> p g l", g=gd),
                            r[:].unsqueeze(1).broadcast_to((128, gd, LHW)),
                        )
                    if MUL_POOL_GROUPS:
                        gp = MUL_POOL_GROUPS
                        nc.gpsimd.tensor_mul(
                            a[:, gd * LHW :].rearrange("p (g l) -> p g l", g=gp),
                            e[:, gd * LHW :].rearrange("p (g l) -> p g l", g=gp),
                            r[:].unsqueeze(1).broadcast_to((128, gp, LHW)),
                        )
                    for j in range(2):
                        for i in range(4):
                            n = 4 * j + i
                            sl = slice(i * LHW, (i + 1) * LHW)
                            nc.tensor.matmul(
                                avp[j][:, sl],
                                vT_sb[n][:, mt * C : (mt + 1) * C],
                                a[:, n * LHW : (n + 1) * LHW],
                                start=(mt == 0 and i % 2 == 0),
                                stop=(mt == NMT - 1 and i % 2 == 1),
                            )
                if mt == NMT - 1:
                    emit_epilogue(avps.pop(lh), lh)

            # One flat software-pipelined stream over all (lh, mt) jobs; the
            # scores stream runs SKEW jobs ahead of softmax/AV, including
            # across the l-half boundary, so no pipeline drain in between.
            jobs = [
                (lh, mt)
                for _ in range(REPEAT)
                for lh in range(NLH)
                for mt in range(NMT)
            ]
            for t, job in enumerate(jobs):
                emit_scores(*job)
                if t >= SKEW:
                    emit_soft_av(*jobs[t - SKEW])
            for job in jobs[len(jobs) - SKEW :]:
                emit_soft_av(*job)


_NC = None


def _get_nc():
    global _NC
    if _NC is None:
        _NC = build()
    return _NC


def kernel(x, Wq, bq, Wk, bk, Wv, bv):
    global LAST_RESULTS
    x = np.ascontiguousarray(np.asarray(x, dtype=np.float32))
    Wq = np.ascontiguousarray(np.asarray(Wq, dtype=np.float32))
    Wk = np.ascontiguousarray(np.asarray(Wk, dtype=np.float32))
    Wv = np.ascontiguousarray(np.asarray(Wv, dtype=np.float32))
    bq = np.asarray(bq, dtype=np.float32).reshape(C, 1)
    bk = np.asarray(bk, dtype=np.float32).reshape(C, 1)
    bv = np.asarray(bv, dtype=np.float32).reshape(1, C)

    xf = x.reshape(N, C, L)
    xflat = x.reshape(N, C * H * W)

    in_maps = []
    for d in range(NCORES):
        lo = d * LSH
        in_maps.append(
            {
                "xk": xf,
                "xq": np.ascontiguousarray(xf[:, :, lo : lo + LSH]),
                "wq": Wq,
                "wk": Wk,
                "wv": Wv,
                "bq": bq,
                "bk": bk,
                "bv": bv,
            }
        )

    nc = _get_nc()
    res = run_bass_kernel_spmd(
        nc, in_maps, core_ids=list(range(NCORES)), trace=TRACE
    )
    LAST_RESULTS = res
    # Device returns attention output in (c, l)-major tiles; reinterleave to
    # the reference's flat (l, c) order and add the residual here.
    att = np.concatenate(
        [
            res.results[d]["out"].transpose(0, 1, 3, 2).reshape(N, LSH * C)
            for d in range(NCORES)
        ],
        axis=1,
    )
    return (xflat + att).reshape(N, C, H, W)



# revision 66
# speedup vs baseline: 1.0151x; 1.0151x over previous
"""Trainium2 Bass kernel for nn_Attention_40570261078258.

Computes, for x:(8,128,64,64), Wq/Wk/Wv:(128,128), bq/bk/bv:(128,):
    xf = x.reshape(N, C, L);  L = 4096
    q/k/v = W @ xf + b                  -> (N, L, C) logical
    scores = q @ k^T / sqrt(C)          -> (N, L, L)
    attn = softmax(scores, axis=0)      # over the BATCH axis (torch legacy dim=0)
    out = attn @ v                      -> (N, L, C)
    return x + out.reshape(N, C, H, W)  # reinterpreting (L,C) memory as (C,H,W)

Sharding: the softmax couples all batch elements at each (l, m) pair, so
batch-parallel would need a 64MB denominator all-reduce. Instead we shard the
query dim L across the 8 cores: each core handles l in [d*512, (d+1)*512) for
ALL batch elements, making the softmax entirely local (no collectives).
Each core redundantly computes k/v for all of L (cheap vs. attention).

SPMD: all cores run the identical graph; the per-core slice is selected by the
host passing a per-core q-input slice (xq). The device returns the attention
output in (c,l)-major tiles; the host reinterleaves and adds the residual.
"""

import math

import numpy as np

import concourse.bacc as bacc
import concourse.bass as bass
import concourse.mybir as mybir
import concourse.tile as tile
from concourse.bass_utils import run_bass_kernel_spmd

N, C, H, W = 8, 128, 64, 64
L = H * W            # 4096 pixels
NCORES = 8
LSH = L // NCORES    # 512 query positions per core
NLH = 2              # l-halves per core
LHW = LSH // NLH     # 256 l per half
NMT = L // 128       # 32 key/value tiles of 128

FP = mybir.dt.float32
FR = mybir.dt.float32r
BF = mybir.dt.bfloat16
AF = mybir.ActivationFunctionType

SKEW = 3           # m-tiles of lookahead between scores and softmax/AV
MUL_POOL_GROUPS = 4   # of the 8 normalize-mul batch groups, how many on gpsimd
REPEAT = 1         # benchmarking: emit the attention phase this many times
MUL_FLAT = False   # normalize-mul as 8 plain 2D ops instead of 3D broadcast
BENCH_INTERNAL = False  # benchmarking: x + out in internal DRAM (no transfer)

# Set by test harness to capture a profile.
TRACE = False
LAST_RESULTS = None


def build():
    nc = bacc.Bacc(
        "TRN2",
        target_bir_lowering=False,
        debug=False,
        enable_asserts=True,
        num_devices=NCORES,
    )

    # x and the transposed weights are declared float32r (same bits as f32)
    # so the projection matmuls run at full PE rate without a bf16 pre-cast.
    if not BENCH_INTERNAL:
        xk = nc.dram_tensor("xk", [N, C, L], FR, kind="ExternalInput").ap()
        xq = nc.dram_tensor("xq", [N, C, LSH], FR, kind="ExternalInput").ap()
    else:
        xk = nc.dram_tensor("xk_i", [N, C, L], FR, kind="Internal").ap()
        xq = nc.dram_tensor("xq_i", [N, C, LSH], FR, kind="Internal").ap()
    # Weights arrive pre-transposed from the host: w*t[c, o] = W[o, c].
    wq = nc.dram_tensor("wqt", [C, C], FR, kind="ExternalInput").ap()
    wk = nc.dram_tensor("wkt", [C, C], FR, kind="ExternalInput").ap()
    wv = nc.dram_tensor("wvt", [C, C], FR, kind="ExternalInput").ap()
    bq = nc.dram_tensor("bq", [C, 1], FP, kind="ExternalInput").ap()
    bk = nc.dram_tensor("bk", [C, 1], FP, kind="ExternalInput").ap()
    bv = nc.dram_tensor("bv", [1, C], FP, kind="ExternalInput").ap()
    # Attention output in (c, l)-major layout; the host does the cheap
    # (l,c) reinterleave + residual add (pure glue, 0.4% of the FLOPs).
    if not BENCH_INTERNAL:
        out = nc.dram_tensor(
            "out", [N, NLH, C, LHW], FP, kind="ExternalOutput"
        ).ap()
    else:
        out = nc.dram_tensor(
            "out_i", [N, NLH, C, LHW], FP, kind="Internal"
        ).ap()
        tok = nc.dram_tensor("tok", [1, 4], FP, kind="ExternalOutput").ap()

    with tile.TileContext(nc) as tc:
        if BENCH_INTERNAL:
            # Zero the internal x so exp() sees sane values; one-time cost,
            # constant across variants (cancels in the repeat slope).
            with tc.tile_pool(name="zinit", bufs=1) as zp:
                zt = zp.tile([128, 2048], FP, tag="z0")
                nc.vector.memset(zt[:], 0.0)
                xkf = xk.rearrange("n c l -> (n c) l").rearrange(
                    "(b p) l -> b p l", p=128
                )
                for b in range(xkf.shape[0]):
                    for c0 in range(0, xkf.shape[2], 2048):
                        nc.sync.dma_start(
                            xkf[b, :, c0 : c0 + 2048].bitcast(FP), zt[:]
                        )
                xqf = xq.rearrange("n c l -> (n c) l").rearrange(
                    "(b p) l -> b p l", p=128
                )
                for b in range(xqf.shape[0]):
                    nc.sync.dma_start(xqf[b].bitcast(FP), zt[:, : xqf.shape[2]])
                nc.sync.dma_start(tok, zt[0:1, 0:4])
        _emit(nc, tc, xk, xq, wq, wk, wv, bq, bk, bv, out)

    nc.compile()
    return nc


def _emit(nc, tc, xk, xq, wq, wk, wv, bq, bk, bv, out):
    from contextlib import ExitStack

    with ExitStack() as ctx:
        cpool = ctx.enter_context(tc.tile_pool(name="const", bufs=1))
        resid = ctx.enter_context(tc.tile_pool(name="resident", bufs=1))

        # --- constants -----------------------------------------------------
        bq_t = cpool.tile([C, 1], FP, tag="bq")
        nc.sync.dma_start(bq_t[:], bq)
        bk_t = cpool.tile([C, 1], FP, tag="bk")
        nc.sync.dma_start(bk_t[:], bk)
        bv_f = cpool.tile([1, C], FP, tag="bvf")
        nc.sync.dma_start(bv_f[:], bv)
        ones_row = cpool.tile([1, C], FP, tag="ones")
        nc.vector.memset(ones_row[:], 1.0)
        # bv replicated across partitions (rank-1 ones @ bv matmul)
        bv_rep = cpool.tile([128, C], FP, tag="bvrep")

        # Pre-transposed weights (WT[c, o] = W[o, c], f32r bits straight
        # from the host); projections run as float32r matmuls (full PE rate
        # at free dim >= 256, ~fp32 precision, no bf16 pre-cast of x).
        wT = {}
        with tc.tile_pool(name="wpsum", bufs=1, space="PSUM") as wpsum_pool:
            for name, wap in (("q", wq), ("k", wk), ("v", wv)):
                wt = cpool.tile([C, C], FR, tag=f"w{name}T")
                nc.sync.dma_start(wt[:], wap)
                wT[name] = wt
            pb = wpsum_pool.tile([128, C], FP, tag="wps")
            nc.tensor.matmul(pb[:], ones_row[:], bv_f[:], start=True, stop=True)
            nc.vector.tensor_copy(bv_rep[:], pb[:])
        # WvT padded to 256 columns of zeros so the float32r vT matmuls hit
        # the >=256 free-dim full-rate path (junk half never read).
        wvpad = cpool.tile([C, 2 * C], FR, tag="wvpad")
        zpad = cpool.tile([C, 2 * C], FP, tag="zpad")
        nc.vector.memset(zpad[:], 0.0)
        nc.vector.tensor_copy(wvpad[:], zpad[:])
        nc.vector.tensor_copy(wvpad[:, 0:C], wT["v"][:])

        # --- resident activations -----------------------------------------
        # q_sb[n]: (c, l) for this core's l-slice;  k_sb[n]: (c, m) full L;
        # vT_sb[n]: (m % 128, 32*128) i.e. 32 chunks of (m,c), all bf16.
        q_sb = [
            resid.tile([C, LSH], BF, tag=f"q{n}", name=f"q_sb{n}") for n in range(N)
        ]
        k_sb = [
            resid.tile([C, L], BF, tag=f"k{n}", name=f"k_sb{n}") for n in range(N)
        ]
        vT_sb = [
            resid.tile([128, NMT * C], BF, tag=f"v{n}", name=f"vT_sb{n}")
            for n in range(N)
        ]

        # --- phase 1: projections (float32r matmuls straight from f32r x) ---
        wqT_r = wT["q"][:]
        wkT_r = wT["k"][:]
        wvpad_r = wvpad[:]
        with (
            tc.tile_pool(name="xin", bufs=3) as xin_pool,
            tc.tile_pool(name="pj", bufs=2, space="PSUM") as pj_psum,
            tc.tile_pool(name="pv", bufs=2, space="PSUM") as pv_psum,
        ):
            for n in range(N):
                # q from the per-core slice
                xt = xin_pool.tile([C, LSH], FR, tag="x")
                nc.sync.dma_start(xt[:], xq[n])
                pq = pj_psum.tile([128, 1024], FP, tag="pj")
                nc.tensor.matmul(
                    pq[:, 0:512], wqT_r, xt[:], start=True, stop=True
                )
                nc.scalar.activation(
                    q_sb[n][:], pq[:, 0:512], AF.Identity, bias=bq_t[:]
                )

                for bch in range(L // 1024):
                    xt = xin_pool.tile([C, 1024], FR, tag="x")
                    nc.sync.dma_start(
                        xt[:], xk[n, :, bch * 1024 : (bch + 1) * 1024]
                    )
                    xr_ = xt[:]

                    # Both 512-wide k matmuls land in one 2-bank PSUM tile so
                    # the bias-adding eviction runs as a single 1024-wide op.
                    pk = pj_psum.tile([128, 1024], FP, tag="pj")
                    for half in range(2):
                        nc.tensor.matmul(
                            pk[:, half * 512 : (half + 1) * 512],
                            wkT_r,
                            xr_[:, half * 512 : (half + 1) * 512],
                            start=True,
                            stop=True,
                        )
                    nc.scalar.activation(
                        k_sb[n][:, bch * 1024 : (bch + 1) * 1024],
                        pk[:],
                        AF.Identity,
                        bias=bk_t[:],
                    )

                    for half in range(2):
                        ch = 2 * bch + half
                        # vT chunks: out[m,c] = sum_c' x[c',m] WvT[c',c] + bv[c]
                        # Each 128-m sub-tile occupies a 256-wide PSUM slice
                        # (f32r full-rate needs >=256 free; upper half junk).
                        # One group per 2KB bank: `start` only on the first
                        # matmul into each bank.
                        pv = pv_psum.tile([128, 1024], FP, tag="pv")
                        for sub in range(4):
                            sl = slice(sub * 256, sub * 256 + 256)
                            nc.tensor.matmul(
                                pv[:, sl],
                                xr_[:, half * 512 + sub * 128 :
                                    half * 512 + (sub + 1) * 128],
                                wvpad_r,
                                start=(sub % 2 == 0),
                                stop=(sub % 2 == 1),
                            )
                        # DVE (idle during phase 1) takes the vT eviction;
                        # strided 3D read picks the real 128 of each 256, and
                        # the bv bias rides along via the broadcast add.
                        nc.vector.scalar_tensor_tensor(
                            vT_sb[n][:, ch * 512 : (ch + 1) * 512].rearrange(
                                "p (s c) -> p s c", s=4
                            ),
                            pv[:].rearrange("p (s c2) -> p s c2", s=4)[
                                :, :, 0:128
                            ],
                            1.0,
                            bv_rep[:].unsqueeze(1).broadcast_to((128, 4, C)),
                            mybir.AluOpType.mult,
                            mybir.AluOpType.add,
                        )

        # --- phase 2: attention with softmax over batch --------------------
        inv_sqrt_c = 1.0 / math.sqrt(C)
        with (
            tc.tile_pool(name="scp", bufs=2, space="PSUM") as sc_psum,
            tc.tile_pool(name="avp", bufs=2, space="PSUM") as av_psum,
            tc.tile_pool(name="soft", bufs=1) as soft_pool,
            tc.tile_pool(name="ost", bufs=1) as ost_pool,
        ):
            def emit_epilogue(avp_prev, lh_prev):
                # Evict PSUM accumulators (freeing the av slots for the next
                # l-half) straight to DRAM in (c, l)-major layout.
                for n in range(N):
                    j, i = n // 4, n % 4
                    ob = ost_pool.tile([128, LHW], FP, tag="ob", bufs=4)
                    nc.vector.tensor_copy(
                        ob[:], avp_prev[j][:, i * LHW : (i + 1) * LHW]
                    )
                    nc.sync.dma_start(out[n, lh_prev], ob[:])

            pend = {}   # (lh, mt) -> E tile (128, 8n x 256l)
            avps = {}   # lh -> accumulator tiles

            def emit_scores(lh, mt):
                l0 = lh * LHW
                e = soft_pool.tile([128, 2048], BF, tag="E", bufs=SKEW + 3)
                for j in range(2):
                    ps = sc_psum.tile([128, 1024], FP, tag="sc")
                    for i in range(4):
                        n = 4 * j + i
                        nc.tensor.matmul(
                            ps[:, i * LHW : (i + 1) * LHW],
                            k_sb[n][:, mt * 128 : (mt + 1) * 128],
                            q_sb[n][:, l0 : l0 + LHW],
                            start=True,
                            stop=True,
                        )
                    nc.scalar.activation(
                        e[:, j * 1024 : (j + 1) * 1024],
                        ps[:],
                        AF.Exp,
                        scale=inv_sqrt_c,
                    )
                pend[(lh, mt)] = e

            def emit_soft_av(lh, mt):
                if mt == 0:
                    # Two (c, 4n x 256l) accumulators, 2 PSUM banks each;
                    # group start/stop is per 2KB bank.
                    avps[lh] = [
                        av_psum.tile(
                            [128, 1024], FP, tag="av", name=f"avp{lh}_{j}"
                        )
                        for j in range(2)
                    ]
                avp = avps[lh]
                e = pend.pop((lh, mt))
                if True:
                    s1 = soft_pool.tile([128, 1024], BF, tag="zt1", bufs=3)
                    nc.vector.tensor_add(s1[:], e[:, 0:1024], e[:, 1024:2048])
                    s2 = soft_pool.tile([128, 512], BF, tag="zt2", bufs=3)
                    nc.vector.tensor_add(s2[:], s1[:, 0:512], s1[:, 512:1024])
                    zr = soft_pool.tile([128, LHW], BF, tag="zr", bufs=4)
                    nc.vector.tensor_add(zr[:], s2[:, 0:LHW], s2[:, LHW : 2 * LHW])
                    r = soft_pool.tile([128, LHW], BF, tag="r", bufs=4)
                    with nc.allow_low_precision(
                        "softmax denom is a sum of 8 O(1..500) exps; bf16 ok"
                    ):
                        nc.vector.reciprocal(r[:], zr[:])
                    # attn[n] = E[n] * (1/Z) via stride-0 broadcast of r along
                    # the batch-group dim; n0-3 on DVE, n4-7 on the otherwise
                    # idle gpsimd (each half feeds its own AV matmuls, so the
                    # slower engine's latency pipelines away).
                    a = soft_pool.tile([128, 2048], BF, tag="A", bufs=SKEW + 1)
                    gd = 8 - MUL_POOL_GROUPS
                    if MUL_FLAT:
                        for g in range(8):
                            eng = nc.vector if g < gd else nc.gpsimd
                            eng.tensor_mul(
                                a[:, g * LHW : (g + 1) * LHW],
                                e[:, g * LHW : (g + 1) * LHW],
                                r[:],
                            )
                    elif gd:
                        nc.vector.tensor_mul(
                            a[:, : gd * LHW].rearrange("p (g l) -> p g l", g=gd),
                            e[:, : gd * LHW].rearrange("p (g l) -> p g l", g=gd),
                            r[:].unsqueeze(1).broadcast_to((128, gd, LHW)),
                        )
                    if MUL_POOL_GROUPS:
                        gp = MUL_POOL_GROUPS
                        nc.gpsimd.tensor_mul(
                            a[:, gd * LHW :].rearrange("p (g l) -> p g l", g=gp),
                            e[:, gd * LHW :].rearrange("p (g l) -> p g l", g=gp),
                            r[:].unsqueeze(1).broadcast_to((128, gp, LHW)),
                        )
                    for j in range(2):
                        for i in range(4):
                            n = 4 * j + i
                            sl = slice(i * LHW, (i + 1) * LHW)
                            nc.tensor.matmul(
                                avp[j][:, sl],
                                vT_sb[n][:, mt * C : (mt + 1) * C],
                                a[:, n * LHW : (n + 1) * LHW],
                                start=(mt == 0 and i % 2 == 0),
                                stop=(mt == NMT - 1 and i % 2 == 1),
                            )
                if mt == NMT - 1:
                    emit_epilogue(avps.pop(lh), lh)

            # One flat software-pipelined stream over all (lh, mt) jobs; the
            # scores stream runs SKEW jobs ahead of softmax/AV, including
            # across the l-half boundary, so no pipeline drain in between.
            jobs = [
                (lh, mt)
                for _ in range(REPEAT)
                for lh in range(NLH)
                for mt in range(NMT)
            ]
            for t, job in enumerate(jobs):
                emit_scores(*job)
                if t >= SKEW:
                    emit_soft_av(*jobs[t - SKEW])
            for job in jobs[len(jobs) - SKEW :]:
                emit_soft_av(*job)


_NC = None


def _get_nc():
    global _NC
    if _NC is None:
        _NC = build()
    return _NC


def kernel(x, Wq, bq, Wk, bk, Wv, bv):
    global LAST_RESULTS
    x = np.ascontiguousarray(np.asarray(x, dtype=np.float32))
    WqT = np.ascontiguousarray(np.asarray(Wq, dtype=np.float32).T)
    WkT = np.ascontiguousarray(np.asarray(Wk, dtype=np.float32).T)
    WvT = np.ascontiguousarray(np.asarray(Wv, dtype=np.float32).T)
    bq = np.asarray(bq, dtype=np.float32).reshape(C, 1)
    bk = np.asarray(bk, dtype=np.float32).reshape(C, 1)
    bv = np.asarray(bv, dtype=np.float32).reshape(1, C)

    xf = x.reshape(N, C, L)
    xflat = x.reshape(N, C * H * W)

    in_maps = []
    for d in range(NCORES):
        lo = d * LSH
        in_maps.append(
            {
                "xk": xf,
                "xq": np.ascontiguousarray(xf[:, :, lo : lo + LSH]),
                "wqt": WqT,
                "wkt": WkT,
                "wvt": WvT,
                "bq": bq,
                "bk": bk,
                "bv": bv,
            }
        )

    nc = _get_nc()
    res = run_bass_kernel_spmd(
        nc, in_maps, core_ids=list(range(NCORES)), trace=TRACE
    )
    LAST_RESULTS = res
    # Device returns attention output in (c, l)-major tiles; reinterleave to
    # the reference's flat (l, c) order and add the residual here.
    att = np.concatenate(
        [
            res.results[d]["out"].transpose(0, 1, 3, 2).reshape(N, LSH * C)
            for d in range(NCORES)
        ],
        axis=1,
    )
    return (xflat + att).reshape(N, C, H, W)

